# revision 9
# baseline (speedup 1.0000x reference)
"""Trainium2 Bass kernel for the 2-layer heterogeneous GCN encoder.

v5 strategy (8 NeuronCores, SPMD, dst-sharded):
  - Core k owns user rows [k*12500,(k+1)*12500) and item rows
    [k*6250,(k+1)*6250); edges are routed to their dst owner.
  - Full fp16 node tables are replicated to every core as inputs (no
    input AllGathers); layer-1 outputs are AllGathered (2 collectives).
  - Aggregate-then-transform: segment_sum(x[src]*norm, dst) @ W with the
    per-window segment sum done as PE matmuls against an on-chip one-hot
    S[e, col] = (dstcol[e] == col) * norm[e] (one 2-op DVE tensor_scalar
    per tile; the DVE instruction stream contains nothing else, so it
    free-runs ahead of the PE).
  - Cross-chunk window accumulation stays on PE/Act: the previous partial
    (fp16 SBUF acc) is re-injected into each fresh PSUM chain with an
    identity matmul, and the chain result is copied psum->acc on the
    Scalar engine. No DVE instruction ever waits on a gather.
  - Flush (feat-major): W matmuls -> Act bias(+relu, scale) -> PE
    transpose -> Act copy -> DMA write (SP). idx loads ride the Pool
    queue so SP's queue only holds output writes.
  - src rows fetched with 1024-row SWDGE dma_gather segments (int16
    indices; >1024 idx/call wedges the SWDGE ucode).

Self-contained: hardcodes shapes; host does index prep (degrees/norms,
sharding, (chunk,window) sort, int16 packing, fp16 casts).
"""

import os
import sys

sys.path.insert(0, "/opt/trn_rl_repo")

import numpy as np

import concourse.bass as bass
import concourse.bacc as bacc
import concourse.mybir as mybir
import concourse.tile as tile
from concourse.bass_utils import run_bass_kernel_spmd

P = 128
NCORES = 8
F32 = mybir.dt.float32
F16 = mybir.dt.float16
I16 = mybir.dt.int16

CFG = dict(N_U=100000, N_I=50000, E=1600000, D=128)
WIN = 256      # dst rows per aggregation window
CHUNK = 32768  # table rows addressable by one int16 gather index space
TSEG = 8       # tiles (x128 rows) per dma_gather call (1024-idx ucode cap)
NQ = 4         # SWDGE queues


def _cdiv(a, b):
    return (a + b - 1) // b


def prep_relation(src, dst, n_src, n_dst, ncores=NCORES, win=WIN, chunk=CHUNK,
                  tseg=TSEG):
    """Shard edges by dst owner, sort by (src-chunk, dst-window), pad each
    (chunk,window) run to whole 128-edge tiles harmonized across cores.

    Returns (sched dict, per-core [idx16, dstw32, norm32])."""
    shard = n_dst // ncores
    nwin = _cdiv(shard, win)
    nchunk = _cdiv(n_src, chunk)

    deg_s = np.bincount(src, minlength=n_src).astype(np.float64)
    deg_d = np.bincount(dst, minlength=n_dst).astype(np.float64)
    inv_s = np.where(deg_s > 0, 1.0 / np.sqrt(deg_s), 0.0)
    inv_d = np.where(deg_d > 0, 1.0 / np.sqrt(deg_d), 0.0)
    norm = (inv_s[src] * inv_d[dst]).astype(np.float32)

    owner = dst // shard
    counts = np.zeros((ncores, nchunk, nwin), np.int64)
    per_core = []
    for k in range(ncores):
        sel = owner == k
        s_k = src[sel]
        d_k = dst[sel] - k * shard
        n_k = norm[sel]
        key = (s_k // chunk) * nwin + (d_k // win)
        order = np.argsort(key, kind="stable")
        s_k, d_k, n_k = s_k[order], d_k[order], n_k[order]
        counts[k] = np.bincount(key[order], minlength=nchunk * nwin).reshape(
            nchunk, nwin
        )
        per_core.append((s_k, d_k, n_k))

    Twc = -(-counts.max(axis=0) // P)          # [nchunk, nwin] tiles
    T_c = Twc.sum(axis=1)                       # tiles per chunk
    base_c = np.concatenate([[0], np.cumsum(T_c)[:-1]])
    base_cw = np.zeros((nchunk, nwin), np.int64)
    flat = Twc.reshape(-1)
    base_cw.reshape(-1)[:] = np.concatenate([[0], np.cumsum(flat)[:-1]])
    Ttot = max(int(T_c.sum()), 1)

    sched = dict(
        nwin=nwin, nchunk=nchunk, shard=shard,
        Twc=Twc.tolist(), T_c=T_c.tolist(), base_c=base_c.tolist(), Ttot=Ttot,
    )

    packed = []
    for k in range(ncores):
        s_k, d_k, n_k = per_core[k]
        idxw = np.zeros((16, Ttot * 8), np.int16)
        dstw = np.full((P, Ttot), -1.0, np.float32)
        nrm = np.zeros((P, Ttot), np.float32)
        cnt = counts[k]
        starts = np.concatenate([[0], np.cumsum(cnt.ravel())[:-1]])
        tok = np.arange(len(s_k)) - np.repeat(starts, cnt.ravel())
        c_e = s_k // chunk
        w_e = d_k // win
        t_stream = base_cw[c_e, w_e] + tok // P    # global stream tile
        p = tok % P
        dstw[p, t_stream] = (d_k % win).astype(np.float32)
        nrm[p, t_stream] = n_k
        t_loc = t_stream - base_c[c_e]
        j = (t_loc % tseg) * P + p                  # position within segment
        seg = t_loc // tseg
        col = (base_c[c_e] + seg * tseg) * 8 + j // 16
        idxw[j % 16, col] = (s_k - c_e * chunk).astype(np.int16)
        packed.append((np.tile(idxw, (8, 1)), dstw, nrm))
    return sched, packed


def build_program(cfg, scheds, win=WIN, chunk=CHUNK, tseg=TSEG):
    N_U, N_I, D = cfg["N_U"], cfg["N_I"], cfg["D"]
    SU, SI = N_U // NCORES, N_I // NCORES
    NWU, NWI = _cdiv(SU, win), _cdiv(SI, win)

    ABL_NOS = os.environ.get("ABL_NOS") == "1"
    ABL_NOGATHER = os.environ.get("ABL_NOGATHER") == "1"
    ABL_SDEC = os.environ.get("ABL_SDEC") == "1"

    nc = bacc.Bacc("TRN2", target_bir_lowering=False, num_swdge_queues=NQ)

    xu_in = nc.dram_tensor("xu16", [N_U, D], F16, kind="ExternalInput")
    xi_in = nc.dram_tensor("xi16", [N_I, D], F16, kind="ExternalInput")
    W16in = {
        n: nc.dram_tensor(f"{n}_h", [D, D], F16, kind="ExternalInput")
        for n in ["W1_follows", "W1_rates", "W1_rev",
                  "W2_follows", "W2_rates", "W2_rev"]
    }
    bs = {
        n: nc.dram_tensor(n, [D], F32, kind="ExternalInput")
        for n in ["b1_follows", "b1_rates", "b1_rev",
                  "b2_follows", "b2_rates", "b2_rev"]
    }
    iota_in = nc.dram_tensor("iota16", [P, win], F16, kind="ExternalInput")
    identh_in = nc.dram_tensor("identF", [P, P], F16, kind="ExternalInput")
    ident_in = nc.dram_tensor("ident", [P, P], F32, kind="ExternalInput")
    streams = {}
    for r, sc in scheds.items():
        streams[r] = dict(
            idx=nc.dram_tensor(f"idx_{r}", [P, sc["Ttot"] * 8], I16,
                               kind="ExternalInput"),
            dstw=nc.dram_tensor(f"dstw_{r}", [P, sc["Ttot"]], F32,
                                kind="ExternalInput"),
            norm=nc.dram_tensor(f"norm_{r}", [P, sc["Ttot"]], F32,
                                kind="ExternalInput"),
        )
    out_user = nc.dram_tensor("out_user", [SU, D], F16, kind="ExternalOutput")
    out_item = nc.dram_tensor("out_item", [SI, D], F16, kind="ExternalOutput")

    qctr = [0]

    def next_q():
        q = qctr[0] % NQ
        qctr[0] += 1
        return q

    with tile.TileContext(nc) as tc:
        with (
            tc.tile_pool(name="const", bufs=1) as cp,
            tc.tile_pool(name="accp", bufs=1) as ap_,
            tc.tile_pool(name="ixp", bufs=2) as ip,
            tc.tile_pool(name="gp", bufs=10) as gp,
            tc.tile_pool(name="Sp", bufs=24) as sp,
            tc.tile_pool(name="hp", bufs=4) as hp,
            tc.tile_pool(name="outp", bufs=6) as outp,
            tc.tile_pool(name="ps", bufs=4, space="PSUM") as pp,
            tc.tile_pool(name="ps2", bufs=2, space="PSUM") as pp2,
            tc.tile_pool(name="pstr", bufs=2, space="PSUM") as ptr,
            tc.tile_pool(name="dram", bufs=1, space="DRAM") as dp,
        ):
            # ---- constants ----
            iota_t = cp.tile([P, win], F16, tag="iota")
            nc.sync.dma_start(iota_t[:], iota_in[:])
            identh_t = cp.tile([P, P], F16, tag="identF")
            nc.sync.dma_start(identh_t[:], identh_in[:])
            ident_t = cp.tile([P, P], F32, tag="ident")
            nc.sync.dma_start(ident_t[:], ident_in[:])
            W_t = {}
            for n, W in W16in.items():
                W_t[n] = cp.tile([P, P], F16, tag=f"W_{n}", name=f"W_{n}")
                nc.sync.dma_start(W_t[n][:], W[:])
            b_t = {}
            for n, b in bs.items():
                b_t[n] = cp.tile([P, 1], F32, tag=f"b_{n}", name=f"bt_{n}")
                nc.sync.dma_start(b_t[n][:], b[:].unsqueeze(1))
            buv = {}
            for l in (1, 2):
                buv[l] = cp.tile([P, 1], F32, tag=f"b{l}uv", name=f"b{l}uv")
                nc.vector.tensor_tensor(
                    out=buv[l][:], in0=b_t[f"b{l}_follows"][:],
                    in1=b_t[f"b{l}_rev"][:], op=mybir.AluOpType.add,
                )
                nc.vector.tensor_scalar_mul(buv[l][:], buv[l][:], 0.5)
            st = {}
            for r, sc in scheds.items():
                st[r] = {}
                for a in ("dstw", "norm"):
                    st[r][a] = cp.tile([P, sc["Ttot"]], F32, tag=f"{a}_{r}",
                                       name=f"{a}t_{r}")
                    nc.sync.dma_start(st[r][a][:], streams[r][a][:])

            # ---- DRAM layer-2 tables ----
            u_slice = dp.tile([SU, D], F16, tag="u_slice")
            it_slice = dp.tile([SI, D], F16, tag="it_slice")
            u_full = dp.tile([N_U, D], F16, tag="u_full", addr_space="Shared")
            it_full = dp.tile([N_I, D], F16, tag="it_full", addr_space="Shared")

            def emit_sweep(rel, table_ap, table_rows, acc, flush_cb=None):
                """(c,w)-major edge sweep for one relation; window partial
                aggregates accumulate into `acc` [P, nwin*win] fp16."""
                sc = scheds[rel]
                nchunk, nwin = sc["nchunk"], sc["nwin"]
                Twc, T_c, base_c = sc["Twc"], sc["T_c"], sc["base_c"]
                last_chunk = [
                    max((c for c in range(nchunk) if Twc[c][w] > 0), default=-1)
                    for w in range(nwin)
                ]
                first = [True] * nwin
                Tcmax = max(T_c)
                for c in range(nchunk):
                    if T_c[c] == 0:
                        continue
                    ixb = ip.tile([P, Tcmax * 8], I16, tag="ix", name="ixb")
                    nc.gpsimd.dma_start(
                        ixb[:, : T_c[c] * 8],
                        streams[rel]["idx"][
                            :, base_c[c] * 8 : (base_c[c] + T_c[c]) * 8
                        ],
                    )
                    cur_seg = -1
                    gbuf = None
                    t_cursor = 0
                    for w in range(nwin):
                        nt = Twc[c][w]
                        if nt == 0:
                            continue
                        ps = pp.tile([P, win], F32, tag="runps")
                        wsl = acc[:, w * win : (w + 1) * win]
                        if not first[w]:
                            # re-inject prior partial into the fresh psum
                            # chain on PE (keeps acc updates off the DVE)
                            nc.tensor.matmul(
                                out=ps[:], lhsT=identh_t[:], rhs=wsl,
                                start=True, stop=False,
                            )
                        for j in range(nt):
                            t_loc = t_cursor + j
                            s = t_loc // tseg
                            if s != cur_seg:
                                cur_seg = s
                                L = min(tseg, T_c[c] - s * tseg)
                                c0 = s * tseg * 8
                                gbuf = gp.tile([P, tseg, P], F16, tag="g")
                                if not ABL_NOGATHER:
                                    nc.gpsimd.dma_gather(
                                        gbuf[:, :L, :],
                                        table_ap[
                                            c * chunk : min((c + 1) * chunk,
                                                            table_rows), :
                                        ],
                                        ixb[:, c0 : c0 + L * 8],
                                        L * P,
                                        L * P,
                                        D,
                                        elem_step=D,
                                        queue_num=next_q(),
                                    )
                            t_glob = base_c[c] + t_loc
                            if ABL_NOS:
                                Sg = iota_t
                            else:
                                Sg = sp.tile([P, win], F16, tag="S")
                                nc.vector.tensor_scalar(
                                    out=Sg[:],
                                    in0=iota_t[:],
                                    scalar1=st[rel]["dstw"][:, t_glob : t_glob + 1],
                                    scalar2=st[rel]["norm"][:, t_glob : t_glob + 1],
                                    op0=mybir.AluOpType.is_equal,
                                    op1=mybir.AluOpType.mult,
                                )
                            nc.tensor.matmul(
                                out=ps[:],
                                lhsT=gbuf[:, t_loc % tseg, :]
                                if not ABL_NOGATHER else iota_t[:, :P],
                                rhs=Sg[:] if not ABL_SDEC else iota_t[:],
                                start=(j == 0 and first[w]),
                                stop=(j == nt - 1),
                            )
                        first[w] = False
                        nc.scalar.activation(
                            out=wsl, in_=ps[:],
                            func=mybir.ActivationFunctionType.Copy,
                        )
                        if flush_cb is not None and c == last_chunk[w]:
                            flush_cb(w)
                        t_cursor += nt
                for w in range(nwin):
                    if last_chunk[w] < 0:
                        nc.vector.memset(acc[:, w * win : (w + 1) * win], 0.0)
                        if flush_cb is not None:
                            flush_cb(w)

            def write_block(h, dst_ap, w, nrows):
                """h [P(feat), win] f32 -> transpose -> dst rows fp16."""
                for blk in range(_cdiv(nrows, P)):
                    r0, r1 = blk * P, min((blk + 1) * P, nrows)
                    pt = ptr.tile([P, P], F32, tag="ptr")
                    nc.tensor.transpose(
                        out=pt[: r1 - r0, :], in_=h[:, r0:r1],
                        identity=ident_t[:],
                    )
                    ob = outp.tile([P, P], F16, tag="ob")
                    nc.scalar.activation(
                        out=ob[: r1 - r0, :], in_=pt[: r1 - r0, :],
                        func=mybir.ActivationFunctionType.Copy,
                    )
                    nc.sync.dma_start(
                        dst_ap[w * win + r0 : w * win + r1, :], ob[: r1 - r0, :]
                    )

            def make_user_flush(l, accF, accV, dst_ap):
                Wf, Wv = W_t[f"W{l}_follows"], W_t[f"W{l}_rev"]
                bias = buv[l]

                def flush(w):
                    nrows = min(win, SU - w * win)
                    wsl = slice(w * win, (w + 1) * win)
                    ph = pp2.tile([P, win], F32, tag="phps")
                    nc.tensor.matmul(out=ph[:], lhsT=Wf[:], rhs=accF[:, wsl],
                                     start=True, stop=False)
                    nc.tensor.matmul(out=ph[:], lhsT=Wv[:], rhs=accV[:, wsl],
                                     start=False, stop=True)
                    h = hp.tile([P, win], F32, tag="h")
                    nc.scalar.activation(
                        out=h[:], in_=ph[:],
                        func=mybir.ActivationFunctionType.Relu if l == 1
                        else mybir.ActivationFunctionType.Identity,
                        bias=bias[:], scale=0.5,
                    )
                    write_block(h, dst_ap, w, nrows)

                return flush

            def make_item_flush(l, accR, dst_ap):
                Wr = W_t[f"W{l}_rates"]
                bias = b_t[f"b{l}_rates"]

                def flush(w):
                    nrows = min(win, SI - w * win)
                    wsl = slice(w * win, (w + 1) * win)
                    ph = pp2.tile([P, win], F32, tag="phps")
                    nc.tensor.matmul(out=ph[:], lhsT=Wr[:], rhs=accR[:, wsl],
                                     start=True, stop=True)
                    h = hp.tile([P, win], F32, tag="h")
                    nc.scalar.activation(
                        out=h[:], in_=ph[:],
                        func=mybir.ActivationFunctionType.Relu if l == 1
                        else mybir.ActivationFunctionType.Identity,
                        bias=bias[:], scale=1.0,
                    )
                    write_block(h, dst_ap, w, nrows)

                return flush

            def user_pass(l, tabU, rowsU, tabI, rowsI, dst_ap):
                accF = ap_.tile([P, NWU * win], F16, tag="accF")
                accV = ap_.tile([P, NWU * win], F16, tag="accV")
                emit_sweep("follows", tabU, rowsU, accF)
                emit_sweep("rev", tabI, rowsI, accV,
                           flush_cb=make_user_flush(l, accF, accV, dst_ap))

            def item_pass(l, tabU, rowsU, dst_ap):
                accR = ap_.tile([P, NWI * win], F16, tag="accR")
                emit_sweep("rates", tabU, rowsU, accR,
                           flush_cb=make_item_flush(l, accR, dst_ap))

            # ---- layer 1 ----
            user_pass(1, xu_in.ap(), N_U, xi_in.ap(), N_I, u_slice)
            nc.gpsimd.collective_compute(
                "AllGather", mybir.AluOpType.bypass,
                replica_groups=[list(range(NCORES))],
                ins=[u_slice[:]], outs=[u_full[:]],
            )
            item_pass(1, xu_in.ap(), N_U, it_slice)
            nc.gpsimd.collective_compute(
                "AllGather", mybir.AluOpType.bypass,
                replica_groups=[list(range(NCORES))],
                ins=[it_slice[:]], outs=[it_full[:]],
            )
            # ---- layer 2 (rates first: only needs u_full) ----
            item_pass(2, u_full, N_U, out_item.ap())
            user_pass(2, u_full, N_U, it_full, N_I, out_user.ap())

    nc.compile()
    return nc


def prepare(inputs, cfg=None, win=WIN, chunk=CHUNK, tseg=TSEG):
    """Host-side prep + program build. Returns (nc, in_maps)."""
    if cfg is None:
        cfg = dict(CFG)
    N_U = inputs["x_user"].shape[0]
    N_I = inputs["x_item"].shape[0]
    cfg.update(N_U=N_U, N_I=N_I, E=len(inputs["follows_src"]))

    rel_edges = {
        "follows": (inputs["follows_src"], inputs["follows_dst"], N_U, N_U),
        "rates": (inputs["rates_src"], inputs["rates_dst"], N_U, N_I),
        "rev": (inputs["rev_src"], inputs["rev_dst"], N_I, N_U),
    }
    scheds, packs = {}, {}
    for r, (s, d, ns, nd) in rel_edges.items():
        sched, packed = prep_relation(
            np.asarray(s), np.asarray(d), ns, nd,
            win=win, chunk=chunk, tseg=tseg,
        )
        scheds[r] = sched
        packs[r] = packed

    nc = build_program(cfg, scheds, win=win, chunk=chunk, tseg=tseg)

    common = {
        "xu16": np.asarray(inputs["x_user"]).astype(np.float16),
        "xi16": np.asarray(inputs["x_item"]).astype(np.float16),
        "iota16": np.broadcast_to(
            np.arange(win, dtype=np.float16), (P, win)
        ).copy(),
        "identF": np.eye(P, dtype=np.float16),
        "ident": np.eye(P, dtype=np.float32),
    }
    for n in ["W1_follows", "W1_rates", "W1_rev",
              "W2_follows", "W2_rates", "W2_rev"]:
        common[f"{n}_h"] = np.asarray(inputs[n]).astype(np.float16)
    for n in ["b1_follows", "b1_rates", "b1_rev",
              "b2_follows", "b2_rates", "b2_rev"]:
        common[n] = np.asarray(inputs[n]).astype(np.float32)

    in_maps = []
    for k in range(NCORES):
        m = dict(common)
        for r in rel_edges:
            idxw, dstw, nrm = packs[r][k]
            m[f"idx_{r}"] = idxw
            m[f"dstw_{r}"] = dstw
            m[f"norm_{r}"] = nrm
        in_maps.append(m)
    return nc, in_maps


def assemble(results):
    u2 = np.concatenate([results[k]["out_user"] for k in range(NCORES)], axis=0)
    i2 = np.concatenate([results[k]["out_item"] for k in range(NCORES)], axis=0)
    return np.concatenate([u2, i2], axis=0).astype(np.float32)


def kernel(**inputs):
    nc, in_maps = prepare(inputs)
    res = run_bass_kernel_spmd(nc, in_maps, list(range(NCORES)))
    return assemble(res.results)


if __name__ == "__main__":
    pass
